# revision 7
# baseline (speedup 1.0000x reference)
"""Trainium2 Bass kernel for nn_ClusterMemory (scatter_memory).

Strategy
--------
Column-shard ("tensor parallel") the three memory banks along num_samples:
core c owns bank columns [c*2048, (c+1)*2048).  Every core receives the full
(l2-normalized, transposed, bf16) student batch and computes its [1024, 2048]
block of the three similarity matrices C_b = x_b @ F_b^T on the PE in bf16.

Loss decomposition (all cross-core combination is a sum of per-core
per-row partial reductions, done on host):

  CE(out_b)    = mean_i [ log(sum_j exp(C/T)) - C[i,t_i]/T ]
                 -> device: row-sums of exp(C/T) via ACT Exp+accum.
                 -> C[i,t_i] = <x_i, f_{t_i}> via per-core row-slice dot with
                    host-gathered target rows (DVE scalar_tensor_tensor+accum).
  MSE ld_b     = sum_d mean_i (x - t)^2  -> device row partials (DVE sub +
                 ACT Square+accum on the core's 128-row slice).
  CE(soft_b)   = mean_i [ log(sum_j exp(s_ij)) - s[i,t_i] ],
                 s = softmax_j(dist).  dist in [0,2] => s_ij <= ~1e-4, so
                 sum_j exp(s_ij) = N + sum_j s_ij + O(sum s^2) = N + 1 + ~3e-5
                 (error ~2e-9 in the log).  Only Zd_i = sum_j exp(dist_ij)
                 is data-dependent, and it only enters through
                 s_t = exp(d_t)/Zd ~ 6e-5, so Zd tolerates ~1e-3 rel error:
                 exp(sqrt(2-2c)) is replaced by its quadratic fit
                 a2*(c+beta)^2 + c0 on the achievable domain of c, evaluated
                 as a single ACT Square (bias=beta) with accum, with the
                 affine applied on host.  This keeps the Scalar engine in the
                 "exp" activation-table set for the whole kernel (no ~2.7us
                 table switches).
"""

import numpy as np
import ml_dtypes

import bass_rust
import concourse.bass as bass
import concourse.tile as tile
from concourse import mybir
from concourse.bass_utils import run_bass_kernel_spmd

B, D, N = 1024, 2048, 16384
TEMP, LAMBDA2, MU = 0.05, 0.5, 1.0
NCORES = 8
JSH = N // NCORES          # 2048 bank columns per core
RSH = B // NCORES          # 128-row slice per core for MSE / target dots
KT = D // 128              # 16 contraction tiles
NJC = 2                    # column chunks of 1024 per core
JCW = JSH // NJC           # 1024
NIT = B // 128             # 8 row tiles
NSLOT = 3 * NJC * NIT      # 48 accumulation slots

BF16 = ml_dtypes.bfloat16

# quadratic fit of f(c) = exp(sqrt(2 - 2c)) on the reachable cosine domain
_c = np.linspace(-0.35, 0.35, 4001)
_a2, _a1, _a0 = np.polyfit(_c, np.exp(np.sqrt(2.0 - 2.0 * _c)), 2)
QBETA = float(_a1 / (2.0 * _a2))          # Square bias
QA2 = float(_a2)                          # host-side scale
QC0 = float(_a0 - _a1 * _a1 / (4.0 * _a2))  # host-side offset

_NC_CACHE = {}
TRACE = False
TRACE_KWARGS = {}
LAST_RESULTS = None
LEGALIZE = True  # CoreSim needs the pre-legalized program; hardware needs it


def _legalize_sync_waits(nc):
    """The walrus build in this container encodes at most one sync wait per
    instruction; hoist extra waits into standalone EventSemaphore sequencer
    instructions on the same engine immediately before the instruction
    (identical semantics: the sequencer blocks before issuing)."""
    f = nc.m.functions[0]
    for blk in f.blocks:
        out = []
        for ins in blk.instructions:
            si = ins.sync_info
            if si is not None:
                waits = list(si.on_wait)
                ups = list(si.on_update or [])
                assert len(ups) <= 1, ins.concise()
                if len(waits) > 1:
                    for w in waits[:-1]:
                        ev = mybir.InstEventSemaphore(
                            name=f"lgw-{nc.next_id()}", ins=[], outs=[])
                        ev.engine = ins.engine
                        ev.sync_info = bass_rust.SyncInfo(on_wait=[w],
                                                          on_update=[])
                        out.append(ev)
                    ins.sync_info = bass_rust.SyncInfo(on_wait=[waits[-1]],
                                                      on_update=ups)
            out.append(ins)
        blk.instructions = out


def _build_nc():
    f32 = mybir.dt.float32
    bf16 = mybir.dt.bfloat16
    nc = bass.Bass("TRN2", target_bir_lowering=False, debug=False,
                   num_devices=NCORES)

    xt_d = [nc.dram_tensor(f"xt{b}", [D, B], bf16, kind="ExternalInput")
            for b in range(3)]
    ft_d = [nc.dram_tensor(f"ft{b}", [D, JSH], bf16, kind="ExternalInput")
            for b in range(3)]
    xs_d = [nc.dram_tensor(f"xs{b}", [RSH, D], bf16, kind="ExternalInput")
            for b in range(3)]
    ts_d = [nc.dram_tensor(f"tn{b}", [RSH, D], bf16, kind="ExternalInput")
            for b in range(3)]
    g_d = [nc.dram_tensor(f"g{b}", [RSH, D], bf16, kind="ExternalInput")
           for b in range(3)]
    zout_o = nc.dram_tensor("zout_o", [128, NSLOT], f32, kind="ExternalOutput")
    sq_o = nc.dram_tensor("sq_o", [128, NSLOT], f32, kind="ExternalOutput")
    ct_o = nc.dram_tensor("ct_o", [128, 3], f32, kind="ExternalOutput")
    ld_o = nc.dram_tensor("ld_o", [128, 3], f32, kind="ExternalOutput")

    with tile.TileContext(nc) as tc:
        with (
            tc.tile_pool(name="xtp", bufs=2) as xt_pool,
            tc.tile_pool(name="ftp", bufs=2) as ft_pool,
            tc.tile_pool(name="scr", bufs=2) as scr_pool,
            tc.tile_pool(name="slp", bufs=1) as sl_pool,
            tc.tile_pool(name="res", bufs=1) as res_pool,
            tc.tile_pool(name="psp", bufs=4, space="PSUM") as ps_pool,
        ):
            zout_sb = res_pool.tile([128, NSLOT], f32, name="zout_sb")
            sq_sb = res_pool.tile([128, NSLOT], f32, name="sq_sb")
            ct_sb = res_pool.tile([128, 3], f32, name="ct_sb")
            ld_sb = res_pool.tile([128, 3], f32, name="ld_sb")
            beta_sb = res_pool.tile([128, 1], f32, name="beta_sb")
            nc.vector.memset(beta_sb, QBETA)

            for b in range(3):
                # --- per-core row-slice work: MSE partials + target dots ---
                xs_t = sl_pool.tile([128, D], bf16, name="xs_t", tag="xs")
                nc.sync.dma_start(out=xs_t, in_=xs_d[b].ap())
                ts_t = sl_pool.tile([128, D], bf16, name="ts_t", tag="ts")
                nc.sync.dma_start(out=ts_t, in_=ts_d[b].ap())
                g_t = sl_pool.tile([128, D], bf16, name="g_t", tag="g")
                nc.sync.dma_start(out=g_t, in_=g_d[b].ap())

                diff_t = sl_pool.tile([128, D], bf16, name="diff_t", tag="diff")
                nc.vector.tensor_sub(diff_t, xs_t, ts_t)
                msescr = sl_pool.tile([128, D], bf16, name="msescr", tag="msescr")
                nc.scalar.activation(msescr, diff_t,
                                     mybir.ActivationFunctionType.Square,
                                     accum_out=ld_sb[:, b:b + 1])
                ctscr = sl_pool.tile([128, D], bf16, name="ctscr", tag="ctscr")
                nc.vector.scalar_tensor_tensor(
                    ctscr, xs_t, 0.0, g_t,
                    op0=mybir.AluOpType.add, op1=mybir.AluOpType.mult,
                    accum_out=ct_sb[:, b:b + 1])

                # --- stationary lhsT: x^T k-tiles ---
                xts = []
                for k in range(KT):
                    xt_t = xt_pool.tile([128, B], bf16, name="xt_t",
                                        tag=f"xt{k}")
                    nc.sync.dma_start(
                        out=xt_t, in_=xt_d[b].ap()[k * 128:(k + 1) * 128, :])
                    xts.append(xt_t)

                for jc in range(NJC):
                    fts = []
                    for k in range(KT):
                        ft_t = ft_pool.tile([128, JCW], bf16, name="ft_t",
                                            tag=f"ft{k}")
                        nc.sync.dma_start(
                            out=ft_t,
                            in_=ft_d[b].ap()[k * 128:(k + 1) * 128,
                                             jc * JCW:(jc + 1) * JCW])
                        fts.append(ft_t)
                    for it in range(NIT):
                        ps = ps_pool.tile([128, JCW], mybir.dt.float32,
                                          name="ps", tag="ps")
                        for k in range(KT):
                            lhsT = xts[k][:, it * 128:(it + 1) * 128]
                            for h in range(2):
                                nc.tensor.matmul(
                                    ps[:, h * 512:(h + 1) * 512],
                                    lhsT,
                                    fts[k][:, h * 512:(h + 1) * 512],
                                    start=(k == 0), stop=(k == KT - 1))
                        idx = (b * NJC + jc) * NIT + it
                        e1 = scr_pool.tile([128, JCW], bf16, name="e1",
                                           tag="e1")
                        nc.scalar.activation(
                            e1, ps, mybir.ActivationFunctionType.Exp,
                            scale=1.0 / TEMP,
                            accum_out=zout_sb[:, idx:idx + 1])
                        sqs = scr_pool.tile([128, JCW], bf16, name="sqs",
                                            tag="sqs")
                        nc.scalar.activation(
                            sqs, ps, mybir.ActivationFunctionType.Square,
                            bias=beta_sb, scale=1.0,
                            accum_out=sq_sb[:, idx:idx + 1])

            nc.sync.dma_start(out=zout_o.ap(), in_=zout_sb)
            nc.sync.dma_start(out=sq_o.ap(), in_=sq_sb)
            nc.sync.dma_start(out=ct_o.ap(), in_=ct_sb)
            nc.sync.dma_start(out=ld_o.ap(), in_=ld_sb)
    if LEGALIZE:
        _legalize_sync_waits(nc)
    return nc


def _l2norm_rows(a):
    n = np.sqrt(np.sum(a.astype(np.float64) ** 2, axis=1, keepdims=True))
    return a / np.maximum(n, 1e-12)


def _prep_in_maps(students, teachers, banks, tgt):
    """Host-side shard prep: l2norm, transpose, bf16 cast, target-row gather.
    Returns (in_maps, xn, g_rows)."""
    xn = [_l2norm_rows(s) for s in students]            # float64 [B, D]
    tn = [_l2norm_rows(t) for t in teachers]
    xt_bf = [np.ascontiguousarray(x.T.astype(np.float32)).astype(BF16)
             for x in xn]                               # [D, B] bf16
    ft_bf = [np.ascontiguousarray(f.T).astype(BF16) for f in banks]  # [D, N]
    g_rows = [f[tgt] for f in banks]                    # [B, D] float32

    in_maps = []
    for c in range(NCORES):
        rs = slice(c * RSH, (c + 1) * RSH)
        m = {}
        for b in range(3):
            m[f"xt{b}"] = xt_bf[b]
            m[f"ft{b}"] = np.ascontiguousarray(
                ft_bf[b][:, c * JSH:(c + 1) * JSH])
            m[f"xs{b}"] = xn[b][rs].astype(np.float32).astype(BF16)
            m[f"tn{b}"] = tn[b][rs].astype(np.float32).astype(BF16)
            m[f"g{b}"] = g_rows[b][rs].astype(BF16)
        in_maps.append(m)
    return in_maps, xn, g_rows


def kernel(inputs, inputs_up, inputs_down, inputs_teacher, inputs_up_teacher,
           inputs_down_teacher, targets, epoch, features, features_up,
           features_down):
    global LAST_RESULTS
    students = [np.asarray(x, np.float32) for x in
                (inputs, inputs_up, inputs_down)]
    teachers = [np.asarray(x, np.float32) for x in
                (inputs_teacher, inputs_up_teacher, inputs_down_teacher)]
    banks = [np.asarray(x, np.float32) for x in
             (features, features_up, features_down)]
    tgt = np.asarray(targets).astype(np.int64)

    in_maps, xn, g_rows = _prep_in_maps(students, teachers, banks, tgt)

    if "nc" not in _NC_CACHE:
        _NC_CACHE["nc"] = _build_nc()
    nc = _NC_CACHE["nc"]

    res = run_bass_kernel_spmd(nc, in_maps, core_ids=list(range(NCORES)),
                               trace=TRACE, **TRACE_KWARGS)
    LAST_RESULTS = res

    # host combine: [128, 48] slot layout is (p, (b, jc, it))
    zout = np.zeros((3, NIT, 128), np.float64)
    sqacc = np.zeros((3, NIT, 128), np.float64)
    ct = np.zeros((3, B), np.float64)
    ld = np.zeros(3, np.float64)
    for c in range(NCORES):
        r = res.results[c]
        zo = r["zout_o"].astype(np.float64).reshape(128, 3, NJC, NIT)
        sq = r["sq_o"].astype(np.float64).reshape(128, 3, NJC, NIT)
        zout += zo.sum(axis=2).transpose(1, 2, 0)
        sqacc += sq.sum(axis=2).transpose(1, 2, 0)
        ct[:, c * RSH:(c + 1) * RSH] = r["ct_o"].astype(np.float64).T
        ld += r["ld_o"].astype(np.float64).sum(axis=0)
    zout = zout.reshape(3, B)    # row i = it*128 + p
    sqacc = sqacc.reshape(3, B)
    ld /= B

    zd = QA2 * sqacc + N * QC0   # sum_j exp(dist_ij), via quadratic surrogate

    loss = 0.0
    weights = [1.0 - LAMBDA2, LAMBDA2, LAMBDA2]
    for b in range(3):
        x2 = np.sum(xn[b] ** 2, axis=1)          # ~1, matches reference cdist
        f2t = np.sum(g_rows[b].astype(np.float64) ** 2, axis=1)
        ce_out = np.mean(np.log(zout[b]) - ct[b] / TEMP)
        d_t = np.sqrt(np.maximum(x2 + f2t - 2.0 * ct[b], 0.0))
        s_t = np.exp(d_t) / zd[b]
        ce_soft = np.log(float(N + 1)) - np.mean(s_t)
        loss += weights[b] * (ce_out + MU * ld[b] + ce_soft)

    return np.float32(loss)


# revision 23
# speedup vs baseline: 18.2752x; 18.2752x over previous
"""Trainium2 Bass kernel for nn_ClusterMemory (scatter_memory).

Strategy
--------
Column-shard ("tensor parallel") the three memory banks along num_samples:
core c owns bank columns [c*2048, (c+1)*2048).  Every core receives the full
(l2-normalized, transposed, bf16) student batch and computes its [1024, 2048]
block of the three similarity matrices C_b = x_b @ F_b^T on the PE in bf16.

Loss decomposition (all cross-core combination is a sum of per-core
per-row partial reductions, done on host):

  CE(out_b)    = mean_i [ log(sum_j exp(C/T)) - C[i,t_i]/T ]
                 -> device: row-sums of exp(C/T) via ACT Exp+accum.
                 -> C[i,t_i] = <x_i, f_{t_i}> via per-core row-slice dot with
                    host-gathered target rows (DVE scalar_tensor_tensor+accum).
  MSE ld_b     = sum_d mean_i (x - t)^2  -> device row partials (DVE sub +
                 ACT Square+accum on the core's 128-row slice).
  CE(soft_b)   = mean_i [ log(sum_j exp(s_ij)) - s[i,t_i] ],
                 s = softmax_j(dist).  dist in [0,2] => s_ij <= ~1e-4, so
                 sum_j exp(s_ij) = N + sum_j s_ij + O(sum s^2) = N + 1 + ~3e-5
                 (error ~2e-9 in the log).  Only Zd_i = sum_j exp(dist_ij)
                 is data-dependent, and it only enters through
                 s_t = exp(d_t)/Zd ~ 6e-5, so Zd tolerates ~1e-3 rel error:
                 exp(sqrt(2-2c)) is replaced by its quadratic fit
                 a2*(c+beta)^2 + c0 on the achievable domain of c, evaluated
                 as a single ACT Square (bias=beta) with accum, with the
                 affine applied on host.  This keeps the Scalar engine in the
                 "exp" activation-table set for the whole kernel (no ~2.7us
                 table switches).
"""

import numpy as np
import ml_dtypes

import bass_rust
import concourse.bass as bass
import concourse.tile as tile
from concourse import mybir
from concourse.bass_utils import run_bass_kernel_spmd

B, D, N = 1024, 2048, 16384
TEMP, LAMBDA2, MU = 0.05, 0.5, 1.0
NCORES = 8
JSH = N // NCORES          # 2048 bank columns per core
RSH = B // NCORES          # 128-row slice per core for MSE / target dots
KT = D // 128              # 16 contraction tiles
NIT = B // 128             # 8 row tiles
NJC = 2                    # j chunks per core (1024 wide each)
JCW = JSH // NJC           # 1024
NSLOT = 3 * NJC * NIT      # 48 accumulation slots

BF16 = ml_dtypes.bfloat16

# quadratic fit of f(c) = exp(sqrt(2 - 2c)) on the reachable cosine domain
_c = np.linspace(-0.35, 0.35, 4001)
_a2, _a1, _a0 = np.polyfit(_c, np.exp(np.sqrt(2.0 - 2.0 * _c)), 2)
QBETA = float(_a1 / (2.0 * _a2))          # Square bias
QA2 = float(_a2)                          # host-side scale
QC0 = float(_a0 - _a1 * _a1 / (4.0 * _a2))  # host-side offset

_NC_CACHE = {}
TRACE = False
TRACE_KWARGS = {}
LAST_RESULTS = None
LEGALIZE = True  # CoreSim needs the pre-legalized program; hardware needs it


def _legalize_sync_waits(nc):
    """The walrus build in this container encodes at most one sync wait per
    instruction; hoist extra waits into standalone EventSemaphore sequencer
    instructions on the same engine immediately before the instruction
    (identical semantics: the sequencer blocks before issuing)."""
    f = nc.m.functions[0]
    for blk in f.blocks:
        out = []
        for ins in blk.instructions:
            si = ins.sync_info
            if si is not None:
                waits = list(si.on_wait)
                ups = list(si.on_update or [])
                assert len(ups) <= 1, ins.concise()
                if len(waits) > 1:
                    for w in waits[:-1]:
                        ev = mybir.InstEventSemaphore(
                            name=f"lgw-{nc.next_id()}", ins=[], outs=[])
                        ev.engine = ins.engine
                        ev.sync_info = bass_rust.SyncInfo(on_wait=[w],
                                                          on_update=[])
                        out.append(ev)
                    ins.sync_info = bass_rust.SyncInfo(on_wait=[waits[-1]],
                                                      on_update=ups)
            out.append(ins)
        blk.instructions = out


def _build_nc(reps=1, skip_act=False, skip_mm=False):
    f32 = mybir.dt.float32
    bf16 = mybir.dt.bfloat16
    nc = bass.Bass("TRN2", target_bir_lowering=False, debug=False,
                   num_devices=NCORES)

    xt_d = [nc.dram_tensor(f"xt{b}", [D, B], bf16, kind="ExternalInput")
            for b in range(3)]
    ft_d = [nc.dram_tensor(f"ft{b}", [D, JSH], bf16, kind="ExternalInput")
            for b in range(3)]
    xs_d = [nc.dram_tensor(f"xs{b}", [RSH, D], bf16, kind="ExternalInput")
            for b in range(3)]
    ts_d = [nc.dram_tensor(f"tn{b}", [RSH, D], bf16, kind="ExternalInput")
            for b in range(3)]
    g_d = [nc.dram_tensor(f"g{b}", [RSH, D], bf16, kind="ExternalInput")
           for b in range(3)]
    zout_o = nc.dram_tensor("zout_o", [128, NSLOT], f32, kind="ExternalOutput")
    sq_o = nc.dram_tensor("sq_o", [128, NSLOT], f32, kind="ExternalOutput")
    ct_o = nc.dram_tensor("ct_o", [128, 3], f32, kind="ExternalOutput")
    ld_o = nc.dram_tensor("ld_o", [128, 3], f32, kind="ExternalOutput")

    with tile.TileContext(nc) as tc:
        with (
            tc.tile_pool(name="xtp", bufs=2) as xt_pool,
            tc.tile_pool(name="ftp", bufs=2) as ft_pool,
            tc.tile_pool(name="scr", bufs=2) as scr_pool,
            tc.tile_pool(name="slp", bufs=1) as sl_pool,
            tc.tile_pool(name="res", bufs=1) as res_pool,
            tc.tile_pool(name="psp", bufs=2, space="PSUM") as ps_pool,
        ):
            import contextlib
            with contextlib.ExitStack() as _rep:
                if reps > 1:  # timing-only: repeat the whole body on-device
                    _rep.enter_context(tc.For_i(0, reps, 1))
                _emit_body(nc, tc, xt_pool, ft_pool, scr_pool, sl_pool,
                           res_pool, ps_pool, xt_d, ft_d, xs_d, ts_d, g_d,
                           zout_o, sq_o, ct_o, ld_o, skip_act, skip_mm)
    if LEGALIZE:
        _legalize_sync_waits(nc)
    return nc


def _emit_body(nc, tc, xt_pool, ft_pool, scr_pool, sl_pool, res_pool,
               ps_pool, xt_d, ft_d, xs_d, ts_d, g_d, zout_o, sq_o, ct_o,
               ld_o, skip_act=False, skip_mm=False):
    f32 = mybir.dt.float32
    bf16 = mybir.dt.bfloat16
    if True:
        if True:
            zout_sb = res_pool.tile([128, NSLOT], f32, name="zout_sb")
            sq_sb = res_pool.tile([128, NSLOT], f32, name="sq_sb")
            ct_sb = res_pool.tile([128, 3], f32, name="ct_sb")
            ld_sb = res_pool.tile([128, 3], f32, name="ld_sb")
            beta_sb = res_pool.tile([128, 1], f32, name="beta_sb")
            nc.vector.memset(beta_sb, QBETA)

            for b in range(3):
                # --- stationary lhsT: all 16 x^T k-tiles in ONE 4MB DMA ---
                xt_big = xt_pool.tile([128, KT, B], bf16, name="xt_big",
                                      tag="xtb")
                nc.sync.dma_start(
                    out=xt_big,
                    in_=xt_d[b].ap().rearrange("(k p) i -> p k i", p=128))

                for jc in range(NJC):
                    # 16 ft k-tiles (1024-wide j chunk) in ONE 4MB DMA
                    ft_big = ft_pool.tile([128, KT, JCW], bf16, name="ft_big",
                                          tag="ftb")
                    nc.sync.dma_start(
                        out=ft_big,
                        in_=ft_d[b].ap().rearrange("(k p) j -> p k j", p=128)[
                            :, :, jc * JCW:(jc + 1) * JCW])
                    for it in range(NIT):
                        ps = ps_pool.tile([128, JCW], mybir.dt.float32,
                                          name="ps", tag="ps")
                        kt_eff = 1 if skip_mm else KT
                        for k in range(kt_eff):
                            lhsT = xt_big[:, k, it * 128:(it + 1) * 128]
                            for h in range(2):
                                nc.tensor.matmul(
                                    ps[:, h * 512:(h + 1) * 512],
                                    lhsT,
                                    ft_big[:, k, h * 512:(h + 1) * 512],
                                    start=(k == 0), stop=(k == kt_eff - 1))
                        idx = (b * NJC + jc) * NIT + it
                        if not skip_act:
                            e1 = scr_pool.tile([128, JCW], bf16, name="e1",
                                               tag="e1")
                            nc.scalar.activation(
                                e1, ps, mybir.ActivationFunctionType.Exp,
                                scale=1.0 / TEMP,
                                accum_out=zout_sb[:, idx:idx + 1])
                            sqs = scr_pool.tile([128, JCW], bf16, name="sqs",
                                                tag="sqs")
                            nc.scalar.activation(
                                sqs, ps, mybir.ActivationFunctionType.Square,
                                bias=beta_sb, scale=1.0,
                                accum_out=sq_sb[:, idx:idx + 1])

                # --- per-core row-slice work: MSE partials + target dots ---
                # (emitted after the matmul stream so the big DMAs go first)
                xs_t = sl_pool.tile([128, D], bf16, name="xs_t", tag="xs")
                nc.sync.dma_start(out=xs_t, in_=xs_d[b].ap())
                ts_t = sl_pool.tile([128, D], bf16, name="ts_t", tag="ts")
                nc.sync.dma_start(out=ts_t, in_=ts_d[b].ap())
                g_t = sl_pool.tile([128, D], bf16, name="g_t", tag="g")
                nc.sync.dma_start(out=g_t, in_=g_d[b].ap())

                diff_t = sl_pool.tile([128, D], bf16, name="diff_t", tag="diff")
                nc.vector.tensor_sub(diff_t, xs_t, ts_t)
                msescr = sl_pool.tile([128, D], bf16, name="msescr", tag="msescr")
                nc.scalar.activation(msescr, diff_t,
                                     mybir.ActivationFunctionType.Square,
                                     accum_out=ld_sb[:, b:b + 1])
                ctscr = sl_pool.tile([128, D], bf16, name="ctscr", tag="ctscr")
                nc.vector.scalar_tensor_tensor(
                    ctscr, xs_t, 0.0, g_t,
                    op0=mybir.AluOpType.add, op1=mybir.AluOpType.mult,
                    accum_out=ct_sb[:, b:b + 1])

            if not skip_act:
                nc.sync.dma_start(out=zout_o.ap(), in_=zout_sb)
                nc.sync.dma_start(out=sq_o.ap(), in_=sq_sb)
            nc.sync.dma_start(out=ct_o.ap(), in_=ct_sb)
            nc.sync.dma_start(out=ld_o.ap(), in_=ld_sb)


def _l2norm_rows(a):
    n = np.sqrt(np.sum(a.astype(np.float64) ** 2, axis=1, keepdims=True))
    return a / np.maximum(n, 1e-12)


def _prep_in_maps(students, teachers, banks, tgt):
    """Host-side shard prep: l2norm, transpose, bf16 cast, target-row gather.
    Returns (in_maps, xn, g_rows)."""
    xn = [_l2norm_rows(s) for s in students]            # float64 [B, D]
    tn = [_l2norm_rows(t) for t in teachers]
    xt_bf = [np.ascontiguousarray(x.T.astype(np.float32)).astype(BF16)
             for x in xn]                               # [D, B] bf16
    ft_bf = [np.ascontiguousarray(f.T).astype(BF16) for f in banks]  # [D, N]
    g_rows = [f[tgt] for f in banks]                    # [B, D] float32

    in_maps = []
    for c in range(NCORES):
        rs = slice(c * RSH, (c + 1) * RSH)
        m = {}
        for b in range(3):
            m[f"xt{b}"] = xt_bf[b]
            m[f"ft{b}"] = np.ascontiguousarray(
                ft_bf[b][:, c * JSH:(c + 1) * JSH])
            m[f"xs{b}"] = xn[b][rs].astype(np.float32).astype(BF16)
            m[f"tn{b}"] = tn[b][rs].astype(np.float32).astype(BF16)
            m[f"g{b}"] = g_rows[b][rs].astype(BF16)
        in_maps.append(m)
    return in_maps, xn, g_rows


def kernel(inputs, inputs_up, inputs_down, inputs_teacher, inputs_up_teacher,
           inputs_down_teacher, targets, epoch, features, features_up,
           features_down):
    global LAST_RESULTS
    students = [np.asarray(x, np.float32) for x in
                (inputs, inputs_up, inputs_down)]
    teachers = [np.asarray(x, np.float32) for x in
                (inputs_teacher, inputs_up_teacher, inputs_down_teacher)]
    banks = [np.asarray(x, np.float32) for x in
             (features, features_up, features_down)]
    tgt = np.asarray(targets).astype(np.int64)

    in_maps, xn, g_rows = _prep_in_maps(students, teachers, banks, tgt)

    if "nc" not in _NC_CACHE:
        _NC_CACHE["nc"] = _build_nc()
    nc = _NC_CACHE["nc"]

    res = run_bass_kernel_spmd(nc, in_maps, core_ids=list(range(NCORES)),
                               trace=TRACE, **TRACE_KWARGS)
    LAST_RESULTS = res

    # host combine: [128, 48] slot layout is (p, (b, jc, it))
    zout = np.zeros((3, NIT, 128), np.float64)
    sqacc = np.zeros((3, NIT, 128), np.float64)
    ct = np.zeros((3, B), np.float64)
    ld = np.zeros(3, np.float64)
    for c in range(NCORES):
        r = res.results[c]
        zo = r["zout_o"].astype(np.float64).reshape(128, 3, NJC, NIT)
        sq = r["sq_o"].astype(np.float64).reshape(128, 3, NJC, NIT)
        zout += zo.sum(axis=2).transpose(1, 2, 0)
        sqacc += sq.sum(axis=2).transpose(1, 2, 0)
        ct[:, c * RSH:(c + 1) * RSH] = r["ct_o"].astype(np.float64).T
        ld += r["ld_o"].astype(np.float64).sum(axis=0)
    zout = zout.reshape(3, B)    # row i = it*128 + p
    sqacc = sqacc.reshape(3, B)
    ld /= B

    zd = QA2 * sqacc + N * QC0   # sum_j exp(dist_ij), via quadratic surrogate

    loss = 0.0
    weights = [1.0 - LAMBDA2, LAMBDA2, LAMBDA2]
    for b in range(3):
        x2 = np.sum(xn[b] ** 2, axis=1)          # ~1, matches reference cdist
        f2t = np.sum(g_rows[b].astype(np.float64) ** 2, axis=1)
        ce_out = np.mean(np.log(zout[b]) - ct[b] / TEMP)
        d_t = np.sqrt(np.maximum(x2 + f2t - 2.0 * ct[b], 0.0))
        s_t = np.exp(d_t) / zd[b]
        ce_soft = np.log(float(N + 1)) - np.mean(s_t)
        loss += weights[b] * (ce_out + MU * ld[b] + ce_soft)

    return np.float32(loss)
